# revision 1
# baseline (speedup 1.0000x reference)
"""ChemConv Bass kernel for 8 TRN2 NeuronCores.

Math: the reference
    node_connection[a,f,i] = sum_n conn[a,n,f] * x[n,i]
    bond_score[a,o,f]      = sum_i node_connection[a,f,i] * pf[o,f,i]
    out[a,o] = sum_f bond_score[a,o,f]*bf[o,f,0] + sum_{f,c} bp[a,f,c]*bf[o,f,1+c]
collapses algebraically to one large matmul plus a small one:
    W[o,f,i]  = pf[o,f,i] * bf[o,f,0]
    Y[k=(n,f), o] = sum_i x[n,i] * W[o,f,i]          (tiny: 24576 x 64)
    out[a,o]  = sum_k conn2d[a,k] * Y[k,o] + sum_j bpT[j,a] * bf2[j,o]
where conn2d = conn.reshape(A, A*F) (201 MB -> the memory-bound stream).

Sharding: atoms (dim a) row-slabs of 256 across 8 cores. Each core computes
out_T[o, a_slab] = Y^T-weighted matmul accumulation over 192 K-chunks of 128.
conn is pre-transposed host-side to [K, a_slab] so DMA loads land with the
contraction dim on SBUF partitions (PE needs partition = K on both operands).
"""

import numpy as np

import concourse.bass as bass
import concourse.tile as tile
from concourse import bacc, mybir
from concourse.bass_utils import run_bass_kernel_spmd

A = 2048
IN_DEPTH = 64
OUT_DEPTH = 64
F = 12
NCORES = 8
AS = A // NCORES          # 256 atoms per core
K = A * F                 # 24576 contraction length
KP = 128                  # K per matmul chunk (partition dim)
KC = K // KP              # 192 chunks
B = 16                    # chunks per DMA batch (16*128*256*4 = 2 MB)
NB = KC // B              # 12 batches
KB = 2 * F                # bond-term contraction length (f,c) = 24

MM_DT = mybir.dt.float32r  # fp32 bits, full-rate PE streaming mode
F32 = mybir.dt.float32

_cache = {}


def _build_nc():
    nc = bacc.Bacc("TRN2", target_bir_lowering=False, debug=False)

    conn_t = nc.dram_tensor("conn_t", [K, AS], MM_DT, kind="ExternalInput").ap()
    ypack = nc.dram_tensor("ypack", [KP, KC * OUT_DEPTH], MM_DT, kind="ExternalInput").ap()
    bond_t = nc.dram_tensor("bond_t", [KB, AS], F32, kind="ExternalInput").ap()
    bf2 = nc.dram_tensor("bf2", [KB, OUT_DEPTH], F32, kind="ExternalInput").ap()
    out_t = nc.dram_tensor("out_t", [OUT_DEPTH, AS], F32, kind="ExternalOutput").ap()

    # DRAM view of conn_t with the chunk partition dim innermost:
    # [K, AS] -> [p=128, nb=KC, a=AS]
    conn_v = conn_t.rearrange("(nb p) a -> p nb a", p=KP)

    with tile.TileContext(nc) as tc:
        with (
            tc.tile_pool(name="const", bufs=1) as cpool,
            tc.tile_pool(name="stream", bufs=3) as spool,
            tc.tile_pool(name="psum", bufs=1, space="PSUM") as ppool,
        ):
            # Y (lhsT chunks), loaded in batch-sized pieces so matmuls can
            # start before the whole 6.3 MB lands.
            y_sb = cpool.tile([KP, KC * OUT_DEPTH], MM_DT)
            y_v = y_sb.rearrange("p (nb o) -> p nb o", nb=KC)
            ydram_v = ypack.rearrange("p (nb o) -> p nb o", nb=KC)
            for i in range(NB):
                nc.sync.dma_start(y_v[:, i * B:(i + 1) * B, :], ydram_v[:, i * B:(i + 1) * B, :])

            bond_sb = cpool.tile([KB, AS], F32)
            nc.sync.dma_start(bond_sb[:], bond_t[:])
            bf2_sb = cpool.tile([KB, OUT_DEPTH], F32)
            nc.sync.dma_start(bf2_sb[:], bf2[:])

            acc = ppool.tile([OUT_DEPTH, AS], F32)

            # bond term opens the PSUM accumulation group (its inputs arrive
            # first; PE can start immediately)
            nc.tensor.matmul(acc[:], bf2_sb[:], bond_sb[:], start=True, stop=False)

            for bt in range(NB):
                ctile = spool.tile([KP, B, AS], MM_DT, tag="conn")
                nc.sync.dma_start(ctile[:], conn_v[:, bt * B:(bt + 1) * B, :])
                for b in range(B):
                    kc = bt * B + b
                    nc.tensor.matmul(
                        acc[:],
                        y_v[:, kc, :],
                        ctile[:, b, :],
                        start=False,
                        stop=(kc == KC - 1),
                    )

            out_sb = cpool.tile([OUT_DEPTH, AS], F32)
            nc.scalar.copy(out_sb[:], acc[:])
            nc.sync.dma_start(out_t[:], out_sb[:])

    nc.compile()
    return nc


def kernel(node_property_tensor, connectivity_tensor, bond_property_tensor,
           property_filters, bond_filters):
    x = np.asarray(node_property_tensor, dtype=np.float32)
    conn = np.asarray(connectivity_tensor, dtype=np.float32)
    bp = np.asarray(bond_property_tensor, dtype=np.float32)
    pf = np.asarray(property_filters, dtype=np.float32)
    bf = np.asarray(bond_filters, dtype=np.float32)

    # Y[k=(n f), o] = sum_i x[n,i] * pf[o,f,i]*bf[o,f,0]
    W = pf * bf[:, :, 0:1]                                # (O, F, I)
    Y = (x @ W.transpose(2, 1, 0).reshape(IN_DEPTH, F * OUT_DEPTH))  # (A, F*O)
    Y2d = Y.reshape(A * F, OUT_DEPTH)                     # k = n*F+f major
    ypack = np.ascontiguousarray(
        Y2d.reshape(KC, KP, OUT_DEPTH).transpose(1, 0, 2).reshape(KP, KC * OUT_DEPTH)
    )

    bf2 = np.ascontiguousarray(bf[:, :, 1:3].reshape(OUT_DEPTH, KB).T)  # (24, O)

    # conn2d^T, per-core row slab of atoms -> [K, AS] contiguous
    conn2dT = np.ascontiguousarray(conn.reshape(A, K).T)  # (K, A)

    if "nc" not in _cache:
        _cache["nc"] = _build_nc()
    nc = _cache["nc"]

    in_maps = []
    for c in range(NCORES):
        sl = slice(c * AS, (c + 1) * AS)
        in_maps.append({
            "conn_t": np.ascontiguousarray(conn2dT[:, sl]),
            "ypack": ypack,
            "bond_t": np.ascontiguousarray(bp[sl].reshape(AS, KB).T),
            "bf2": bf2,
        })

    res = run_bass_kernel_spmd(nc, in_maps, core_ids=list(range(NCORES)))

    out = np.empty((A, OUT_DEPTH), dtype=np.float32)
    for c in range(NCORES):
        out[c * AS:(c + 1) * AS, :] = res.results[c]["out_t"].T
    return out


# revision 5
# speedup vs baseline: 2.1634x; 2.1634x over previous
"""ChemConv Bass kernel for 8 TRN2 NeuronCores.

Math: the reference
    node_connection[a,f,i] = sum_n conn[a,n,f] * x[n,i]
    bond_score[a,o,f]      = sum_i node_connection[a,f,i] * pf[o,f,i]
    out[a,o] = sum_f bond_score[a,o,f]*bf[o,f,0] + sum_{f,c} bp[a,f,c]*bf[o,f,1+c]
collapses algebraically to one large matmul plus a small one:
    W[o,f,i]  = pf[o,f,i] * bf[o,f,0]
    Y[k=(n,f), o] = sum_i x[n,i] * W[o,f,i]          (tiny: 24576 x 64)
    out[a,o]  = sum_k conn2d[a,k] * Y[k,o] + sum_j bpT[j,a] * bf2[j,o]
where conn2d = conn.reshape(A, A*F) (201 MB -> the memory-bound stream).

Sharding: atoms (dim a) row-slabs of 256 across 8 cores. Each core computes
out_T[o, a_slab] = Y^T-weighted matmul accumulation over 192 K-chunks of 128.
conn is pre-transposed host-side to [K, a_slab] so DMA loads land with the
contraction dim on SBUF partitions (PE needs partition = K on both operands).
"""

import numpy as np

import concourse.bass as bass
import concourse.tile as tile
from concourse import bacc, mybir
from concourse.bass_utils import run_bass_kernel_spmd

A = 2048
IN_DEPTH = 64
OUT_DEPTH = 64
F = 12
NCORES = 8
AS = A // NCORES          # 256 atoms per core
K = A * F                 # 24576 contraction length
KP = 128                  # K per matmul chunk (partition dim)
KC = K // KP              # 192 chunks
B = 16                    # chunks per DMA batch (16*128*256*4 = 2 MB)
NB = KC // B              # 12 batches
KB = 2 * F                # bond-term contraction length (f,c) = 24

MM_DT = mybir.dt.float32r  # fp32 bits, full-rate PE streaming mode
F32 = mybir.dt.float32

_cache = {}


def _build_nc(repeat=1):
    """Build the per-core kernel. `repeat` re-runs the conn-streaming body
    N times (benchmark-only: lets wall-clock slope isolate HW time from the
    ~95ms axon dispatch floor). The deliverable path uses repeat=1."""
    nc = bacc.Bacc("TRN2", target_bir_lowering=False, debug=False)

    conn_t = nc.dram_tensor("conn_t", [K, AS], MM_DT, kind="ExternalInput").ap()
    ypack = nc.dram_tensor("ypack", [KP, KC * OUT_DEPTH], MM_DT, kind="ExternalInput").ap()
    bond_t = nc.dram_tensor("bond_t", [KB, AS], F32, kind="ExternalInput").ap()
    bf2 = nc.dram_tensor("bf2", [KB, OUT_DEPTH], F32, kind="ExternalInput").ap()
    out_t = nc.dram_tensor("out_t", [OUT_DEPTH, AS], F32, kind="ExternalOutput").ap()

    # DRAM view of conn_t with the chunk partition dim innermost:
    # [K, AS] -> [p=128, nb=KC, a=AS]
    conn_v = conn_t.rearrange("(nb p) a -> p nb a", p=KP)

    with tile.TileContext(nc) as tc:
        with (
            tc.tile_pool(name="const", bufs=1) as cpool,
            tc.tile_pool(name="stream", bufs=3) as spool,
            tc.tile_pool(name="psum", bufs=2, space="PSUM") as ppool,
        ):
            # Y (lhsT chunks), loaded in batch-sized pieces so matmuls can
            # start before the whole 6.3 MB lands.
            y_sb = cpool.tile([KP, KC * OUT_DEPTH], MM_DT)
            y_v = y_sb.rearrange("p (nb o) -> p nb o", nb=KC)
            ydram_v = ypack.rearrange("p (nb o) -> p nb o", nb=KC)
            for i in range(NB):
                nc.sync.dma_start(y_v[:, i * B:(i + 1) * B, :], ydram_v[:, i * B:(i + 1) * B, :])

            bond_sb = cpool.tile([KB, AS], F32)
            nc.sync.dma_start(bond_sb[:], bond_t[:])
            bf2_sb = cpool.tile([KB, OUT_DEPTH], F32)
            nc.sync.dma_start(bf2_sb[:], bf2[:])

            for rep in range(repeat):
                acc = ppool.tile([OUT_DEPTH, AS], F32, tag="acc")

                # bond term opens the PSUM accumulation group (its inputs
                # arrive first; PE can start immediately)
                nc.tensor.matmul(acc[:], bf2_sb[:], bond_sb[:], start=True, stop=False)

                for bt in range(NB):
                    ctile = spool.tile([KP, B, AS], MM_DT, tag="conn")
                    nc.sync.dma_start(ctile[:], conn_v[:, bt * B:(bt + 1) * B, :])
                    for b in range(B):
                        kc = bt * B + b
                        nc.tensor.matmul(
                            acc[:],
                            y_v[:, kc, :],
                            ctile[:, b, :],
                            start=False,
                            stop=(kc == KC - 1),
                        )

                out_sb = spool.tile([OUT_DEPTH, AS], F32, tag="osb")
                nc.scalar.copy(out_sb[:], acc[:])
                nc.sync.dma_start(out_t[:], out_sb[:])

    nc.compile()
    return nc


def kernel(node_property_tensor, connectivity_tensor, bond_property_tensor,
           property_filters, bond_filters):
    x = np.asarray(node_property_tensor, dtype=np.float32)
    conn = np.asarray(connectivity_tensor, dtype=np.float32)
    bp = np.asarray(bond_property_tensor, dtype=np.float32)
    pf = np.asarray(property_filters, dtype=np.float32)
    bf = np.asarray(bond_filters, dtype=np.float32)

    # Y[k=(n f), o] = sum_i x[n,i] * pf[o,f,i]*bf[o,f,0]
    W = pf * bf[:, :, 0:1]                                # (O, F, I)
    Y = (x @ W.transpose(2, 1, 0).reshape(IN_DEPTH, F * OUT_DEPTH))  # (A, F*O)
    Y2d = Y.reshape(A * F, OUT_DEPTH)                     # k = n*F+f major
    ypack = np.ascontiguousarray(
        Y2d.reshape(KC, KP, OUT_DEPTH).transpose(1, 0, 2).reshape(KP, KC * OUT_DEPTH)
    )

    bf2 = np.ascontiguousarray(bf[:, :, 1:3].reshape(OUT_DEPTH, KB).T)  # (24, O)

    # conn2d^T, per-core row slab of atoms -> [K, AS] contiguous
    conn2dT = np.ascontiguousarray(conn.reshape(A, K).T)  # (K, A)

    if "nc" not in _cache:
        _cache["nc"] = _build_nc()
    nc = _cache["nc"]

    in_maps = []
    for c in range(NCORES):
        sl = slice(c * AS, (c + 1) * AS)
        in_maps.append({
            "conn_t": np.ascontiguousarray(conn2dT[:, sl]),
            "ypack": ypack,
            "bond_t": np.ascontiguousarray(bp[sl].reshape(AS, KB).T),
            "bf2": bf2,
        })

    res = run_bass_kernel_spmd(nc, in_maps, core_ids=list(range(NCORES)))

    out = np.empty((A, OUT_DEPTH), dtype=np.float32)
    for c in range(NCORES):
        out[c * AS:(c + 1) * AS, :] = res.results[c]["out_t"].T
    return out
